# revision 19
# baseline (speedup 1.0000x reference)
"""Trainium2 Bass kernel for nn_CircuitBuilder (topk_masking).

For X [524288, 64] f32, gate_weights [64, 130], output_weights [64, 8],
output_scale [8]:

    buf = [X | 0 | 1 | gate slots]
    top2[i] = top-2 of softmax(gate_weights[i, :66+i])  (data-independent)
    g_i = 1 - a*b   (continuous NAND chain)
    out = (gate_matrix @ output_weights) * output_scale

Design (pure data parallel over 8 cores, 65536 samples each):
  - Host computes the gate wiring and compiles a per-wiring device program.
  - Per-core layout: partition p owns 512 consecutive samples, pair-major:
    vt[p, t, g, u] = gate-slot g at sample 2t+u (bf16). X arrives pre-packed
    the same way with only the used columns (possibly reordered/duplicated).
  - The gate DAG is shallow (4 levels). Per level the a*b products run as
    batched tensor_tensor ops (2x DVE mode): xx gates are chained into
    Euler trails over the column graph (one op per trail via overlapping
    A/B windows), gx gates batch over constant-stride parent runs. The
    "1-x" corrections run as one ranged tensor_scalar op (4x mode) per
    level; slots also covered by a range store 1-m and the projection
    weights absorb the affine flip per slot.
  - Output projection needs v transposed to gate-on-partition. Two paths:
      * PE path: f32-bitcast [128,128] transposes (two bf16 sample pairs
        per f32) -> PSUM -> bf16 drain (Act/Pool) -> matmul.
      * XBAR path: dma_start_transpose maps in[p, 128k+r] -> out[r, k, p];
        the pair-major inner 128-block is exactly (g,u), so the DMA engines
        transpose SBUF->SBUF with no PSUM round-trip.
  - Projection matmuls [16,1024] stack 8-deep into [128,1024] f32 PSUM
    groups -> one bf16 drain per group -> DMA out. Bias + final f32 are
    applied on host during decode.
"""

import hashlib

import numpy as np
import ml_dtypes

N_SAMPLES = 524288
N_FEATURES = 64
N_GATES = 64
N_OUTPUTS = 8
BASE = N_FEATURES + 2            # 66
MAX_CONN = BASE + N_GATES        # 130
N_CORES = 8
N_LOC = N_SAMPLES // N_CORES     # 65536 samples per core
P = 128
NT = N_LOC // (2 * P)            # 256 sample-pairs per partition

# schedule configuration
CHUNKS = [64, 64, 64, 64]        # DVE chunk sizes in pairs (sum == NT)
XB_UNITS = [2, 3, 4, 4]          # trailing 8-pair units per chunk on XBAR
IN_SLICES = 4                    # input DMA granularity (slices of NT)
N_UNITS = NT // 8                # 32 projection units of 8 pairs
# 4 units stack into one [128,1024] f32 PSUM group at partition offsets
# 32*{0..3} (PE tile positions allow only multiples of 32); rows 16..31 of
# each 32-block are unused.
N_GROUPS = N_UNITS // 4          # 8 po groups

assert sum(CHUNKS) == NT and all(c % 8 == 0 for c in CHUNKS)


def _top2(gate_weights: np.ndarray) -> np.ndarray:
    """Top-2 connection indices per gate (matches jax.lax.top_k of the
    softmax: softmax is monotonic, ties break to lower index)."""
    top2 = np.zeros((N_GATES, 2), dtype=np.int64)
    for i in range(N_GATES):
        row = np.asarray(gate_weights[i], dtype=np.float32).copy()
        row[BASE + i:] = -1e9
        top2[i] = np.argsort(-row, kind="stable")[:2]
    return top2


def _build_plan(gate_weights, output_weights, output_scale):
    top2 = _top2(gate_weights)
    W = np.asarray(output_weights, dtype=np.float64)
    scale = np.asarray(output_scale, dtype=np.float64)

    # ---- classify gates -------------------------------------------------
    def kind_of(c):
        if c < N_FEATURES:
            return "x"
        if c == N_FEATURES:
            return "0"
        if c == N_FEATURES + 1:
            return "1"
        return "g"

    gates = {}
    for i in range(N_GATES):
        c0, c1 = int(top2[i][0]), int(top2[i][1])
        k0, k1 = kind_of(c0), kind_of(c1)
        pri = {"g": 0, "x": 1, "1": 2, "0": 3}
        if pri[k0] > pri[k1]:
            c0, c1, k0, k1 = c1, c0, k1, k0
        kk = k0 + k1
        if "0" in kk:
            gates[i] = {"op": "zero"}
        elif kk == "x1":
            gates[i] = {"op": "copyx", "c": c0}
        elif kk == "11":
            gates[i] = {"op": "one"}
        elif kk == "g1":
            gates[i] = {"op": "g1", "p": c0 - BASE}
        elif kk == "xx":
            gates[i] = {"op": "xx", "a": c0, "b": c1}
        elif kk == "gx":
            gates[i] = {"op": "gx", "p": c0 - BASE, "c": c1}
        elif kk == "gg":
            gates[i] = {"op": "gg", "p": c0 - BASE, "q": c1 - BASE}
        else:
            raise AssertionError(f"unexpected operand kinds {kk}")

    def parents(i):
        d = gates[i]
        return [d[k] for k in ("p", "q") if k in d]

    level = {}
    for i in range(N_GATES):
        ps = parents(i)
        level[i] = (1 + max(level[p] for p in ps)) if ps else 0
    max_level = max(level.values())

    consumers = {i: [] for i in range(N_GATES)}
    for i in range(N_GATES):
        for p in parents(i):
            consumers[p].append(i)
    consumed = {i for i in range(N_GATES) if consumers[i]}

    lv_gates = {l: [i for i in range(N_GATES) if level[i] == l]
                for l in range(max_level + 1)}

    # ---- slot assignment -------------------------------------------------
    # L0: xx gates ordered by first consumer (so the next level's parent
    # slots ascend stride-1), then copyx, then zero/one (adjacent memsets).
    # For gg consumers, both parents share the consumer key and land
    # adjacent (tiebreak: p before q), giving dp=dq=2 runs.
    def fc_key(i):
        cons = [(level[j], j, 0 if gates[j].get("p") == i else 1)
                for j in consumers[i]]
        return (min(cons) if cons else (99, i, 0), i)

    slot_of = {}
    gate_of = {}
    s = 0
    ordkind = {"xx": 0, "copyx": 1, "zero": 2, "one": 2}
    l0 = sorted(lv_gates[0], key=lambda i: (ordkind[gates[i]["op"]], fc_key(i)))
    for i in l0:
        slot_of[i] = s
        s += 1
    # L1+: gx sorted by parent slot, then gg (by parent pair), then g1
    for l in range(1, max_level + 1):
        def key(i):
            d = gates[i]
            ps = sorted(slot_of[d[k]] for k in ("p", "q") if k in d)
            return ({"gx": 0, "gg": 1, "g1": 2}[d["op"]], ps[0], ps[-1], i)
        for i in sorted(lv_gates[l], key=key):
            slot_of[i] = s
            s += 1
    assert s == N_GATES
    for i, s2 in slot_of.items():
        gate_of[s2] = i

    # ---- column placement + ops -----------------------------------------
    colmap = []
    col_pos = {}

    def place(c, fresh=False):
        if not fresh and c in col_pos:
            return col_pos[c]
        pos = len(colmap)
        colmap.append(c)
        if c not in col_pos:
            col_pos[c] = pos
        return pos

    ops = []
    corrected = set()             # slots storing g = 1-m

    for l in range(max_level + 1):
        gl = sorted(lv_gates[l], key=lambda i: slot_of[i])

        idx = 0
        while idx < len(gl):
            i = gl[idx]
            d = gates[i]
            op = d["op"]
            if op == "zero":
                ops.append({"k": "memset", "s0": slot_of[i], "n": 1,
                            "val": 0.0})
                idx += 1
            elif op == "one":
                ops.append({"k": "memset", "s0": slot_of[i], "n": 1,
                            "val": 1.0})
                idx += 1
            elif op == "copyx":
                run = [i]
                jdx = idx + 1
                while jdx < len(gl) and gates[gl[jdx]]["op"] == "copyx":
                    run.append(gl[jdx])
                    jdx += 1
                c0 = None
                for g2 in run:
                    pos = place(gates[g2]["c"], fresh=len(run) > 1)
                    c0 = pos if c0 is None else c0
                ops.append({"k": "copyx", "s0": slot_of[run[0]],
                            "n": len(run), "c0": c0})
                idx = jdx
            elif op == "g1":
                ops.append({"k": "copyv", "s0": slot_of[i], "n": 1,
                            "p0": slot_of[d["p"]]})
                idx += 1
            elif op == "xx":
                # one batched op: A then B regions, every ref placed fresh
                run = [i]
                jdx = idx + 1
                while jdx < len(gl) and gates[gl[jdx]]["op"] == "xx":
                    run.append(gl[jdx])
                    jdx += 1
                if len(run) >= 2:
                    a0 = len(colmap)
                    for g2 in run:
                        place(gates[g2]["a"], fresh=True)
                    b0 = len(colmap)
                    for g2 in run:
                        place(gates[g2]["b"], fresh=True)
                    ops.append({"k": "tt", "s0": slot_of[run[0]], "ds": 1,
                                "n": len(run),
                                "in0": ("x", a0, 1), "in1": ("x", b0, 1)})
                else:
                    ops.append({"k": "tt", "s0": slot_of[i], "ds": 1, "n": 1,
                                "in0": ("x", place(d["a"]), 1),
                                "in1": ("x", place(d["b"]), 1)})
                idx = jdx
            elif op == "gx":
                # run: const parent stride; cols dup-placed to keep going
                run = [i]
                jdx = idx + 1
                dstride = None
                while jdx < len(gl):
                    nd = gates[gl[jdx]]
                    if nd["op"] != "gx":
                        break
                    step = slot_of[nd["p"]] - slot_of[gates[run[-1]]["p"]]
                    if step <= 0 or (dstride is not None and step != dstride):
                        break
                    dstride = step
                    run.append(gl[jdx])
                    jdx += 1
                if len(run) == 2 and any(gates[g2]["c"] in col_pos
                                         for g2 in run):
                    run = run[:1]
                if len(run) >= 2:
                    c0 = None
                    for g2 in run:
                        pos = place(gates[g2]["c"], fresh=True)
                        c0 = pos if c0 is None else c0
                    ops.append({"k": "tt", "s0": slot_of[run[0]], "ds": 1,
                                "n": len(run),
                                "in0": ("v", slot_of[d["p"]], dstride),
                                "in1": ("x", c0, 1)})
                    idx = jdx
                else:
                    ops.append({"k": "tt", "s0": slot_of[i], "ds": 1, "n": 1,
                                "in0": ("v", slot_of[d["p"]], 1),
                                "in1": ("x", place(d["c"]), 1)})
                    idx += 1
            elif op == "gg":
                run = [i]
                jdx = idx + 1
                dp = dq = None
                while jdx < len(gl):
                    nd = gates[gl[jdx]]
                    if nd["op"] != "gg":
                        break
                    sp = slot_of[nd["p"]] - slot_of[gates[run[-1]]["p"]]
                    sq = slot_of[nd["q"]] - slot_of[gates[run[-1]]["q"]]
                    if sp <= 0 or sq <= 0:
                        break
                    if dp is not None and (sp != dp or sq != dq):
                        break
                    dp, dq = sp, sq
                    run.append(gl[jdx])
                    jdx += 1
                if len(run) >= 2:
                    ops.append({"k": "tt", "s0": slot_of[run[0]], "ds": 1,
                                "n": len(run),
                                "in0": ("v", slot_of[d["p"]], dp),
                                "in1": ("v", slot_of[d["q"]], dq)})
                    idx = jdx
                else:
                    ops.append({"k": "tt", "s0": slot_of[i], "ds": 1, "n": 1,
                                "in0": ("v", slot_of[d["p"]], 1),
                                "in1": ("v", slot_of[d["q"]], 1)})
                    idx += 1
            else:
                raise AssertionError(op)

        # corrections: runs over consumed slots, bridging gaps <= 3
        cons = sorted(slot_of[i] for i in gl if i in consumed)
        while cons:
            lo = hi = cons.pop(0)
            while cons and cons[0] - hi <= 3:
                hi = cons.pop(0)
            ops.append({"k": "corr", "s0": lo, "n": hi - lo + 1})
            corrected.update(range(lo, hi + 1))

    # leaf singleton products (outputs never consumed) can run on GpSimd,
    # off the serial DVE chain
    cons_slots = {slot_of[i] for i in consumed}
    for op in ops:
        if op["k"] == "tt" and op["n"] <= 2 and not any(
                s2 in cons_slots for s2 in range(op["s0"],
                                                 op["s0"] + op["ds"] * op["n"],
                                                 op["ds"])):
            op["eng"] = "pool"
        else:
            op["eng"] = "dve"

    # ---- projection constants -------------------------------------------
    wsig = np.zeros((N_GATES, N_OUTPUTS))
    bias = np.zeros(N_OUTPUTS)
    for s2 in range(N_GATES):
        wrow = W[gate_of[s2]] * scale
        if s2 in corrected:
            wsig[s2] = wrow
        else:
            wsig[s2] = -wrow
            bias += wrow

    # PE path rows: r = t'*64 + g
    wt = np.zeros((P, 2 * N_OUTPUTS))
    for tp in range(2):
        wt[tp * N_GATES:(tp + 1) * N_GATES,
           tp * N_OUTPUTS:(tp + 1) * N_OUTPUTS] = wsig
    # XBAR path rows: r = 2g + u
    wu = np.zeros((P, 2 * N_OUTPUTS))
    for u in range(2):
        wu[u::2, u * N_OUTPUTS:(u + 1) * N_OUTPUTS] = wsig

    return {
        "gates": gates,
        "ops": ops,
        "colmap": np.array(colmap, dtype=np.int64),
        "ncols": len(colmap),
        "slot_of": slot_of,
        "gate_of": gate_of,
        "consumed": consumed,
        "corrected": corrected,
        "level": level,
        "wt_bf16": wt.astype(ml_dtypes.bfloat16),
        "wu_bf16": wu.astype(ml_dtypes.bfloat16),
        "bias_f32": bias.astype(np.float32),
    }


def _emulate_vt(plan, xt):
    """Emulate the device gate program on sample-major xt [n, ncols] bf16.
    Returns V [n, 64] bf16 slot values."""
    bf = ml_dtypes.bfloat16
    n = xt.shape[0]
    V = np.zeros((n, N_GATES), dtype=bf)
    for op in plan["ops"]:
        k, s0, nn = op["k"], op["s0"], op["n"]
        if k == "memset":
            V[:, s0:s0 + nn] = op["val"]
        elif k == "copyx":
            V[:, s0:s0 + nn] = xt[:, op["c0"]:op["c0"] + nn]
        elif k == "copyv":
            V[:, s0:s0 + nn] = V[:, op["p0"]:op["p0"] + nn]
        elif k == "tt":
            def rd(spec):
                src, o0, ds = spec
                arr = xt if src == "x" else V
                return arr[:, o0:o0 + ds * nn:ds].astype(np.float32)
            V[:, s0:s0 + nn] = (rd(op["in0"]) * rd(op["in1"])).astype(bf)
        elif k == "corr":
            m = V[:, s0:s0 + nn].astype(np.float32)
            V[:, s0:s0 + nn] = (1.0 - m).astype(bf)
        else:
            raise AssertionError(k)
    return V


def _schedule():
    """Derive per-chunk unit lists. Unit k covers pairs [8k, 8k+8)."""
    sched = []
    u0 = 0
    for ci, cp in enumerate(CHUNKS):
        nu = cp // 8
        xb = XB_UNITS[ci]
        sched.append({
            "t0": sum(CHUNKS[:ci]),
            "tn": cp,
            "units": list(range(u0, u0 + nu)),
            "pe_units": list(range(u0, u0 + nu - xb)),
            "xb_units": list(range(u0 + nu - xb, u0 + nu)),
        })
        u0 += nu
    return sched


def _build_bass_kernel(plan):
    import concourse.bacc as bacc
    import concourse.tile as tile
    import concourse.mybir as mybir
    from concourse import masks

    f32 = mybir.dt.float32
    bf16 = mybir.dt.bfloat16
    mult = mybir.AluOpType.mult
    add = mybir.AluOpType.add

    C = plan["ncols"]
    sched = _schedule()

    nc = bacc.Bacc(None, target_bir_lowering=False)
    x_d = nc.dram_tensor("xg", [P, NT, C, 2], bf16, kind="ExternalInput")
    wt_d = nc.dram_tensor("wt", [P, 2 * N_OUTPUTS], bf16, kind="ExternalInput")
    wu_d = nc.dram_tensor("wu", [P, 2 * N_OUTPUTS], bf16, kind="ExternalInput")
    out_d = nc.dram_tensor("out", [N_GROUPS, P, 1024], bf16,
                           kind="ExternalOutput")

    with tile.TileContext(nc) as tc:
        with (
            tc.tile_pool(name="const", bufs=1) as cpool,
            tc.tile_pool(name="xp", bufs=1) as xpool,
            tc.tile_pool(name="vp", bufs=4) as vpool,
            tc.tile_pool(name="txp", bufs=2) as txpool,
            tc.tile_pool(name="vsp", bufs=4) as vspool,
            tc.tile_pool(name="stgp", bufs=2) as stgpool,
            tc.tile_pool(name="ptp", bufs=2, space="PSUM") as ptpool,
            tc.tile_pool(name="pop", bufs=3, space="PSUM") as popool,
        ):
            ident = cpool.tile([128, 128], f32)
            masks.make_identity(nc, ident[:])
            wt_sb = cpool.tile([P, 2 * N_OUTPUTS], bf16)
            nc.sync.dma_start(wt_sb[:], wt_d[:])
            wu_sb = cpool.tile([P, 2 * N_OUTPUTS], bf16)
            nc.sync.dma_start(wu_sb[:], wu_d[:])

            xt = xpool.tile([P, NT, C, 2], bf16)
            tsl = NT // IN_SLICES
            for si in range(IN_SLICES):
                nc.sync.dma_start(xt[:, si * tsl:(si + 1) * tsl, :, :],
                                  x_d[:, si * tsl:(si + 1) * tsl, :, :])

            po_tiles = {}
            po_filled = {}
            ndrain = 0

            for ci, sc in enumerate(sched):
                t0, tn = sc["t0"], sc["tn"]
                vt = vpool.tile([P, tn, N_GATES, 2], bf16, tag="vt")

                def vsl(s0, ds, n):
                    if ds == 1:
                        return vt[:, :, s0:s0 + n, :]
                    return vt[:, :, s0:s0 + ds * n:ds, :]

                def xsl(c0, dc, n):
                    if dc == 1:
                        return xt[:, t0:t0 + tn, c0:c0 + n, :]
                    return xt[:, t0:t0 + tn, c0:c0 + dc * n:dc, :]

                for op in plan["ops"]:
                    k, s0, n = op["k"], op["s0"], op["n"]
                    if k == "memset":
                        nc.gpsimd.memset(vsl(s0, 1, n), op["val"])
                    elif k == "copyx":
                        nc.vector.tensor_copy(vsl(s0, 1, n),
                                              xsl(op["c0"], 1, n))
                    elif k == "copyv":
                        nc.vector.tensor_copy(vsl(s0, 1, n),
                                              vsl(op["p0"], 1, n))
                    elif k == "tt":
                        def rd(spec):
                            src, o0, ds = spec
                            return (xsl(o0, ds, n) if src == "x"
                                    else vsl(o0, ds, n))
                        eng = (nc.gpsimd if op.get("eng") == "pool"
                               else nc.vector)
                        eng.tensor_tensor(
                            vsl(s0, op["ds"], n), rd(op["in0"]),
                            rd(op["in1"]), mult)
                    elif k == "corr":
                        nc.vector.tensor_scalar(
                            vsl(s0, 1, n), vsl(s0, 1, n), -1.0, 1.0,
                            mult, add)
                    else:
                        raise AssertionError(k)

                # ---- output phase ----
                if sc["xb_units"]:
                    xb_p0 = (sc["xb_units"][0] * 8 - t0)   # local pair
                    xb_np = len(sc["xb_units"]) * 8
                    txt = txpool.tile([P, xb_np, P], bf16, tag="txt")
                    nc.sync.dma_start_transpose(
                        txt[:], vt[:, xb_p0:xb_p0 + xb_np, :, :])

                def get_po(k):
                    g = k // 4
                    if g not in po_tiles:
                        po_tiles[g] = popool.tile([P, 1024], f32, name="po",
                                                  tag="po")
                        po_filled[g] = 0
                    return po_tiles[g]

                vb = vt[:].bitcast(f32).rearrange("p t g z -> p (t g z)")
                for k in sc["pe_units"]:
                    po = get_po(k)
                    pt = ptpool.tile([P, 512], f32, tag="pt")
                    for kk in range(4):
                        k2 = (8 * k - t0) // 2 + kk     # local pair-pair
                        nc.tensor.transpose(
                            pt[:, 128 * kk:128 * (kk + 1)],
                            vb[:, 128 * k2:128 * (k2 + 1)], ident[:])
                    vs = vspool.tile([P, 1024], bf16, tag="vs")
                    nc.scalar.copy(vs[:], pt[:].bitcast(bf16))
                    ndrain += 1
                    m = k % 4
                    for h in range(2):
                        nc.tensor.matmul(
                            po[32 * m:32 * m + 16, 512 * h:512 * h + 512],
                            wt_sb[:], vs[:, 512 * h:512 * h + 512],
                            start=True, stop=True, tile_position=(0, 32 * m))
                    po_filled[k // 4] += 1

                for k in sc["xb_units"]:
                    po = get_po(k)
                    lp = 8 * k - sc["xb_units"][0] * 8
                    m = k % 4
                    for h in range(2):
                        rhs = txt[:, lp + 4 * h:lp + 4 * h + 4, :]
                        nc.tensor.matmul(
                            po[32 * m:32 * m + 16, 512 * h:512 * h + 512],
                            wu_sb[:], rhs.rearrange("r k p -> r (k p)"),
                            start=True, stop=True, tile_position=(0, 32 * m))
                    po_filled[k // 4] += 1

                for g in sorted(po_tiles):
                    if po_filled[g] == 4:
                        stg = stgpool.tile([P, 1024], bf16, tag="stg")
                        nc.scalar.copy(stg[:], po_tiles[g][:])
                        nc.gpsimd.dma_start(out_d[g], stg[:])
                        del po_tiles[g]
                        po_filled[g] = -1

    nc.compile()
    return nc


def _decode_idx():
    """dst[g, r, q] -> flat index into [N_LOC, 8]."""
    sched = _schedule()
    is_xb = {}
    xb_base = {}
    for sc in sched:
        for k in sc["pe_units"]:
            is_xb[k] = False
        for k in sc["xb_units"]:
            is_xb[k] = True
            xb_base[k] = sc["xb_units"][0]
    dst = np.full((N_GROUPS, P, 1024), -1, dtype=np.int64)
    for k in range(N_UNITS):
        g, m = k // 4, k % 4
        for rr in range(16):
            half, o = rr // N_OUTPUTS, rr % N_OUTPUTS
            r = 32 * m + rr
            if not is_xb[k]:
                kk = np.arange(4)[:, None, None]
                p_ = np.arange(P)[None, :, None]
                u = np.arange(2)[None, None, :]
                q = (kk * 256 + p_ * 2 + u).reshape(-1)
                j = (2 * (8 * k + 2 * kk + half) + u + 0 * p_).reshape(-1)
                p_f = (0 * kk + p_ + 0 * u).reshape(-1)
            else:
                tl = np.arange(8)[:, None]
                p_ = np.arange(P)[None, :]
                q = (tl * 128 + p_).reshape(-1)
                j = (2 * (8 * k + tl) + half + 0 * p_).reshape(-1)
                p_f = (0 * tl + p_).reshape(-1)
            dst[g, r, q] = (p_f * 512 + j) * N_OUTPUTS + o
    return dst


_DST = None


def _decode_out(dev_out, plan):
    """[N_GROUPS, P, 1024] bf16 device output -> [N_LOC, 8] f32 w/ bias."""
    global _DST
    if _DST is None:
        _DST = _decode_idx()
    flat = np.empty(N_LOC * N_OUTPUTS, dtype=np.float32)
    mask = _DST.reshape(-1) >= 0
    flat[_DST.reshape(-1)[mask]] = np.asarray(dev_out).astype(
        np.float32).reshape(-1)[mask]
    out = flat.reshape(N_LOC, N_OUTPUTS)
    out += plan["bias_f32"][None, :]
    return out


def make_in_maps(X, plan):
    colmap = plan["colmap"]
    # xg[core][p, t, c, u] = X[core*N_LOC + p*512 + 2t + u, colmap[c]]
    arr = np.asarray(X, dtype=np.float32).reshape(N_CORES, P, NT, 2, N_FEATURES)
    xg = arr[..., colmap].transpose(0, 1, 2, 4, 3).astype(ml_dtypes.bfloat16)
    in_maps = []
    for c in range(N_CORES):
        in_maps.append({
            "xg": np.ascontiguousarray(xg[c]),
            "wt": plan["wt_bf16"],
            "wu": plan["wu_bf16"],
        })
    return in_maps


_CACHE = {}


def _get_compiled(gate_weights, output_weights, output_scale):
    key = hashlib.sha256(
        np.asarray(gate_weights, np.float32).tobytes()
        + np.asarray(output_weights, np.float32).tobytes()
        + np.asarray(output_scale, np.float32).tobytes()
    ).hexdigest()
    if key not in _CACHE:
        plan = _build_plan(gate_weights, output_weights, output_scale)
        nc = _build_bass_kernel(plan)
        _CACHE[key] = (plan, nc)
    return _CACHE[key]


def kernel(X, gate_weights, output_weights, output_scale):
    X = np.asarray(X, dtype=np.float32)
    plan, nc = _get_compiled(gate_weights, output_weights, output_scale)
    in_maps = make_in_maps(X, plan)

    from concourse.bass_utils import run_bass_kernel_spmd
    res = run_bass_kernel_spmd(nc, in_maps, list(range(N_CORES)))
    out = np.concatenate(
        [_decode_out(res.results[c]["out"], plan) for c in range(N_CORES)],
        axis=0)
    return out.astype(np.float32)
